# revision 11
# baseline (speedup 1.0000x reference)
"""Trainium2 Bass kernel for nn_Attention (dense transformer block without
head split: qkv proj -> full-width attention over S=2048 -> out proj).

Sharding: 8 cores = 4 batches x 2 query-halves. Each core gets its batch's
full x (token-rotated so its own 1024 queries are rows 0..1023) and computes
attention + output projection for its 1024 queries. No collectives.

Algebraic restructure vs the direct form: the k-projection is eliminated by
folding M = SCALE * (W_q @ W_k^T) on the host, so
    dots = (x @ W_q) @ (x @ W_k)^T * SCALE = (x @ M) @ x^T,
and the v/out projections are folded into w_vo = W_v @ W_out, so
    out = softmax(dots) @ x @ w_vo + b.
Per-core PE work drops from ~17.2 GFLOP (q,k proj + QK + PV + out) to
~12.9 GFLOP (q' proj + QK + PV + out).

x is shipped twice in different layouts (pure host-side marshaling, like the
token rotation / bf16 cast): d-major f32 (xT, the QK lhsT — avoids 129 PE
transposes and their PSUM-drain stalls) and token-major bf16 (xb, PV lhsT).

Precision: QK chain (x, M, q') in f32r, PV/out-proj in bf16 (rel err ~2.5e-3).

Layout (per core):
  xT    [d, t]  f32r  DMA'd directly (host-transposed); QK lhsT
  qT    [d, s]  f32r  q'^T = M^T x^T, rhs for QK (s free, 512-wide)
  xb    [t, d]  bf16  PV lhsT (x doubles as values)
  dotsT [t, s]  psum  QK accumulated over d; ACT exp -> PT bf16 (no max
                      subtraction: logits bounded far below f32 range)
  softmax sums via ones-matmul over the partition dim; sums scattered to
  [128,4] via tiny K=1 matmuls then reciprocal'd per-partition; 1/sum and
  bias are fused into the final evict. Evictions alternate vector/scalar
  engines so the PE never waits on PSUM drains.
"""

import numpy as np

import concourse.mybir as mybir
import concourse.tile as tile
from concourse import bacc
from concourse.bass_utils import run_bass_kernel_spmd

f32 = mybir.dt.float32
f32r = mybir.dt.float32r
bf16 = mybir.dt.bfloat16
AF = mybir.ActivationFunctionType

P = 128
B, S, D = 4, 2048, 1024
INNER = 1024
SQ = S // 2  # queries per core
SCALE = (INNER // 16) ** -0.5  # dim_head=64 -> 0.125

DC = D // P  # 8 d-chunks
TT = S // P  # 16 token tiles
SB = SQ // 512  # 2 query s-blocks per core
N_CORES = 8


def build_nc():
    nc = bacc.Bacc(None, target_bir_lowering=False, dynamic_dma_scratch_size=2048)
    x_tr = nc.dram_tensor("x_tr", [D, S], f32r, kind="ExternalInput")
    x_bf = nc.dram_tensor("x_bf", [S, D], bf16, kind="ExternalInput")
    m = nc.dram_tensor("m", [D, D], f32r, kind="ExternalInput")
    w_vo = nc.dram_tensor("w_vo", [D, D], bf16, kind="ExternalInput")
    b_out = nc.dram_tensor("b_out", [1, D], f32, kind="ExternalInput")
    out = nc.dram_tensor("out", [SQ, D], f32, kind="ExternalOutput")

    xtr_t = x_tr.rearrange("(dc p) t -> p dc t", p=P)  # [128, 8, 2048] (part=d)
    xbf_t = x_bf.rearrange("(tt p) d -> p tt d", p=P)  # [128, 16, 1024]
    m_t = m.rearrange("(dc p) f -> p dc f", p=P)  # [128, 8, 1024] (part=d_in)
    wvo_t = w_vo.rearrange("(dc p) f -> p dc f", p=P)  # [128, 8, 1024] (part=d)

    with tile.TileContext(nc, pool_alloc_mode="queue") as tc:
        with (
            tc.tile_pool(name="persist", bufs=1) as persist,
            tc.tile_pool(name="consts", bufs=1) as consts,
        ):
            xT = persist.tile([P, DC, S], f32r)  # 64K/part
            qT = persist.tile([P, DC, SQ], f32r)  # 32K/part
            xb = persist.tile([P, TT, D], bf16)  # 32K/part (token-major x)
            wvo_bf = persist.tile([P, DC, D], bf16)  # 16K/part

            ones_bf = consts.tile([P, 1], bf16)
            ones_f1 = consts.tile([1, 1], f32)
            ones_row = consts.tile([1, P], f32)
            b_row = consts.tile([1, D], f32)
            bias_bc = consts.tile([P, D], f32)
            sum_sb = consts.tile([1, SB, 512], f32)
            rcp_sp = consts.tile([P, SB, 4], f32)

            nc.sync.dma_start(out=b_row, in_=b_out[:, :])
            nc.vector.memset(ones_bf, 1.0)
            nc.vector.memset(ones_f1, 1.0)
            nc.vector.memset(ones_row, 1.0)

            # xT block 0a (tokens 0..255) first: the very first q'T
            # half-group is gated on only 1.5 MB (blk0a + m chunk 0)
            nc.sync.dma_start(out=xT[:, :, 0:256], in_=xtr_t[:, :, 0:256])

            # ---------------- Phase A: q' projection ----------------
            with (
                tc.tile_pool(name="pa_sbuf", bufs=1) as pa,
                tc.tile_pool(name="pa_psum", bufs=1, space="PSUM") as pap,
            ):
                m_sb = pa.tile([P, DC, D], f32r)  # 32K/part

                def dma_m(k):
                    # m in column-chunks so q'T do-groups unblock incrementally
                    nc.sync.dma_start(
                        out=m_sb[:, :, k * P : (k + 1) * P],
                        in_=m_t[:, :, k * P : (k + 1) * P],
                    )

                dma_m(0)
                nc.sync.dma_start(out=xT[:, :, 256:512], in_=xtr_t[:, :, 256:512])
                for k in range(1, DC):
                    dma_m(k)
                for blk in range(1, 4):
                    nc.sync.dma_start(
                        out=xT[:, :, blk * 512 : (blk + 1) * 512],
                        in_=xtr_t[:, :, blk * 512 : (blk + 1) * 512],
                    )
                for tt in range(TT):
                    nc.sync.dma_start(out=xb[:, tt], in_=xbf_t[:, tt])
                nc.sync.dma_start(out=wvo_bf, in_=wvo_t)

                with nc.named_scope("proj"):
                    # bias broadcast doubles as PE warm-up:
                    # ones[1,128].T @ b_row -> [128, D]
                    for dc2 in range(2):
                        bb_ps = pap.tile([P, 512], f32, tag="kq", bufs=4)
                        nc.tensor.matmul(
                            bb_ps, ones_row, b_row[:, dc2 * 512 : (dc2 + 1) * 512],
                            start=True, stop=True,
                        )
                        nc.vector.tensor_copy(
                            bias_bc[:, dc2 * 512 : (dc2 + 1) * 512], bb_ps
                        )

                    def qproj_group(do, s0, sn):
                        ps = pap.tile([P, 512], f32, tag="kq", bufs=4)
                        for di in range(DC):
                            nc.tensor.matmul(
                                ps[:, :sn],
                                m_sb[:, di, do * P : (do + 1) * P],
                                xT[:, di, s0 : s0 + sn],
                                start=(di == 0),
                                stop=(di == DC - 1),
                            )
                        if do % 2 == 0:
                            nc.vector.tensor_copy(qT[:, do, s0 : s0 + sn], ps[:, :sn])
                        else:
                            nc.scalar.copy(qT[:, do, s0 : s0 + sn], ps[:, :sn])

                    # first group split in half: gated on blk0a + m chunk 0 only
                    qproj_group(0, 0, 256)
                    qproj_group(0, 256, 256)
                    for do in range(1, DC):
                        qproj_group(do, 0, 512)
                    for do in range(DC):
                        qproj_group(do, 512, 512)

            # ---------------- Phase B: attention + out proj ----------------
            with (
                tc.tile_pool(name="pb_sbuf", bufs=1) as pb,
                tc.tile_pool(name="pb_psum", bufs=1, space="PSUM") as pbp,
            ):
                PTs = [None, None]

                def qk_block(sb):
                    with nc.named_scope(f"qk_{sb}"):
                        PT = pb.tile([P, TT, 512], bf16, tag="PT", bufs=2)
                        PTs[sb] = PT
                        for tt in range(TT):
                            dots = pbp.tile([P, 512], f32, tag="dots", bufs=3)
                            for dc in range(DC):
                                nc.tensor.matmul(
                                    dots,
                                    xT[:, dc, tt * P : (tt + 1) * P],
                                    qT[:, dc, sb * 512 : (sb + 1) * 512],
                                    start=(dc == 0),
                                    stop=(dc == DC - 1),
                                )
                            nc.scalar.activation(PT[:, tt, :], dots, AF.Exp)
                    with nc.named_scope(f"sum_{sb}"):
                        sum_ps = pbp.tile([1, 512], f32, tag="small", bufs=1)
                        for tt in range(TT):
                            nc.tensor.matmul(
                                sum_ps,
                                ones_bf,
                                PTs[sb][:, tt, :],
                                start=(tt == 0),
                                stop=(tt == TT - 1),
                            )
                        nc.vector.tensor_copy(sum_sb[:, sb], sum_ps)

                def scatter_rcp(sb):
                    # sums [1,512] -> per-partition [128,4], reciprocal'd; one
                    # psum tile + one reciprocal so the matmuls don't serialize
                    # behind per-column reads
                    with nc.named_scope(f"scat_{sb}"):
                        scat_ps = pbp.tile([P, 4], f32, tag="small", bufs=1)
                        for j in range(4):
                            nc.tensor.matmul(
                                scat_ps[:, j : j + 1],
                                sum_sb[0:1, sb, j * P : (j + 1) * P],
                                ones_f1,
                                start=True,
                                stop=True,
                            )
                        nc.vector.reciprocal(rcp_sp[:, sb], scat_ps)

                pxTs = [None, None]

                def pv_block(sb):
                    with nc.named_scope(f"pv_{sb}"):
                        pxT = pb.tile([P, DC, 512], bf16, tag="pxT", bufs=2)
                        pxTs[sb] = pxT
                        for dc in range(DC):
                            pv_ps = pbp.tile([P, 512], f32, tag="pv", bufs=2)
                            for tt in range(TT):
                                nc.tensor.matmul(
                                    pv_ps,
                                    xb[:, tt, dc * P : (dc + 1) * P],
                                    PTs[sb][:, tt, :],
                                    start=(tt == 0),
                                    stop=(tt == TT - 1),
                                )
                            if dc % 2 == 0:
                                nc.vector.tensor_copy(pxT[:, dc], pv_ps)
                            else:
                                nc.scalar.copy(pxT[:, dc], pv_ps)

                def fin_group(sb, ss, f0, fn):
                    fin_ps = pbp.tile([P, 512], f32, tag="fin", bufs=2)
                    for dc in range(DC):
                        nc.tensor.matmul(
                            fin_ps[:, :fn],
                            pxTs[sb][:, dc, ss * P : (ss + 1) * P],
                            wvo_bf[:, dc, f0 : f0 + fn],
                            start=(dc == 0),
                            stop=(dc == DC - 1),
                        )
                    fin_sb = pb.tile([P, 512], f32, tag="fin_sb", bufs=4)
                    nc.vector.scalar_tensor_tensor(
                        out=fin_sb[:, :fn],
                        in0=fin_ps[:, :fn],
                        scalar=rcp_sp[:, sb, ss : ss + 1],
                        in1=bias_bc[:, f0 : f0 + fn],
                        op0=mybir.AluOpType.mult,
                        op1=mybir.AluOpType.add,
                    )
                    r0 = sb * 512 + ss * P
                    nc.sync.dma_start(
                        out=out[r0 : r0 + P, f0 : f0 + fn], in_=fin_sb[:, :fn]
                    )

                def fin_block(sb):
                    with nc.named_scope(f"fin_{sb}"):
                        for ss in range(4):
                            for dc2 in range(2):
                                last = sb == 1 and ss == 3 and dc2 == 1
                                if last:
                                    # split the final group so the last output
                                    # DMA starts ~1us earlier
                                    fin_group(sb, ss, 512, 256)
                                    fin_group(sb, ss, 768, 256)
                                else:
                                    fin_group(sb, ss, dc2 * 512, 512)

                qk_block(0)
                qk_block(1)
                pv_block(0)
                scatter_rcp(0)
                pv_block(1)
                scatter_rcp(1)
                fin_block(0)
                fin_block(1)

    nc.compile()
    return nc


_NC_CACHE = {}


def _get_nc():
    if "nc" not in _NC_CACHE:
        _NC_CACHE["nc"] = build_nc()
    return _NC_CACHE["nc"]


def _prep_weights(W_qkv, W_out, b_out):
    import ml_dtypes

    W_qkv = np.asarray(W_qkv, dtype=np.float32)
    wq = W_qkv[:, :INNER].astype(np.float64)
    wk = W_qkv[:, INNER : 2 * INNER].astype(np.float64)
    m = np.ascontiguousarray((SCALE * (wq @ wk.T)).astype(np.float32))
    w_vo_f = W_qkv[:, 2 * INNER :].astype(np.float64) @ np.asarray(
        W_out, dtype=np.float32
    ).astype(np.float64)
    w_vo = np.ascontiguousarray(w_vo_f.astype(np.float32).astype(ml_dtypes.bfloat16))
    b = np.ascontiguousarray(np.asarray(b_out, dtype=np.float32)).reshape(1, D)
    return m, w_vo, b


def make_in_maps(x, W_qkv, W_out, b_out):
    import ml_dtypes

    x = np.asarray(x, dtype=np.float32)
    m, w_vo, b = _prep_weights(W_qkv, W_out, b_out)
    in_maps = []
    for c in range(N_CORES):
        bi, h = divmod(c, 2)
        xb = x[bi]
        x_c = np.concatenate([xb[SQ * h :], xb[: SQ * h]], axis=0) if h else xb
        in_maps.append(
            {
                "x_tr": np.ascontiguousarray(x_c.T),
                "x_bf": np.ascontiguousarray(x_c.astype(ml_dtypes.bfloat16)),
                "m": m,
                "w_vo": w_vo,
                "b_out": b,
            }
        )
    return in_maps


def kernel(x, W_qkv, W_out, b_out):
    nc = _get_nc()
    in_maps = make_in_maps(x, W_qkv, W_out, b_out)
    res = run_bass_kernel_spmd(nc, in_maps, core_ids=list(range(N_CORES)))
    full = np.empty((B, S, D), dtype=np.float32)
    for c in range(N_CORES):
        bi, h = divmod(c, 2)
        full[bi, SQ * h : SQ * (h + 1)] = res.results[c]["out"]
    return full
